# revision 30
# baseline (speedup 1.0000x reference)
"""AdaPT Linear (int8 systolic fake-quant matmul) on 8 TRN2 NeuronCores.

Reference semantics (single device):
    amax_x = max|x|, amax_w = max|w|         (global scalars)
    sx = 127/amax_x, sw = 127/amax_w
    qx = round(x*sx)  (int8), qw = round(w*sw)  (int8)
    out = (qx @ qw.T)_int32 / (sx*sw) + bias

Distribution: data-parallel over x rows (8 x 1024 rows per core).

Pipeline per core (one NEFF, Tile generates all semaphores):
  - amax: x-shard partials first (the x path has the longest latency chain),
    then the wT-slice partials; each exchanged via a tiny AllGather; scales =
    127/amax via DVE reciprocal + one Newton step.
  - quantization = fp32 magic-number round (v*s + 1.5*2^23, subtract back;
    fp32 RNE makes this bit-exact round-half-to-even, matching jnp.round).
  - x: natural [m, k] 128-row strips so matmuls unlock per strip: DVE pass1
    (in place) -> ACT pass2 (int8-valued bf16) -> PE 128x128 bf16 transposes
    -> PSUM -> ACT copy into resident qxT [128k, kt, m].
  - w: the host stages wT (k-major), so w quantization is pure vector work:
    per 512-column output block, DVE pass1 + ACT pass2 straight into
    triple-buffered qwT tiles.  No PE transposes for w.
  - matmul: lhsT = qxT k-tile [128k x 128m], rhs = qwT k-tile [128k x 512n],
    32-step accumulation into fp32 PSUM.  int8 products (<2^14) and sums
    (<2^24) are exact in the bf16 PE datapath, reproducing the int8 MAC.
  - epilogue: out = psum * (1/(sx*sw)) + bias in one DVE op, DMA out.
"""

import numpy as np

P = 128
MAGIC = 12582912.0  # 1.5 * 2**23: fp32 RNE round-to-int trick
MAXV = 127.0
NCORES = 8

# full-problem shapes (hardcoded per the task)
FULL_B, FULL_S, FULL_K = 4, 2048, 4096
FULL_N = 4096


def build_graph(M=1024, N=4096, K=4096, ncores=NCORES):
    """Build the SPMD Bass graph for one core (identical on all cores)."""
    import concourse.bass as bass
    import concourse.mybir as mybir
    import concourse.tile as tile
    from concourse import bacc, bass_isa
    from concourse.masks import make_identity

    assert M % P == 0 and K % P == 0 and N % 512 == 0
    KT = K // P             # k tiles
    MB = M // P             # m blocks (x strips)
    NB = N // 512           # n blocks of 512
    KSL = K // ncores       # k-rows of wT per core for amax
    XG = 8                  # k-tiles per x-transpose PSUM group

    f32 = mybir.dt.float32
    bf16 = mybir.dt.bfloat16

    nc = bacc.Bacc(None, num_devices=ncores)

    x_ext = nc.declare_dram_parameter("x", [M, K], f32, isOutput=False)
    wt_ext = nc.declare_dram_parameter("wT", [K, N], f32, isOutput=False)
    wslt_ext = nc.declare_dram_parameter("wslT", [KSL, N], f32, isOutput=False)
    b_ext = nc.declare_dram_parameter("bias", [N], f32, isOutput=False)
    out_ext = nc.declare_dram_parameter("out", [M, N], f32, isOutput=True)

    ccw_in = nc.dram_tensor("ccw_in", [1, 1], f32)
    ccw_out = nc.dram_tensor("ccw_out", [ncores, 1], f32)
    ccx_in = nc.dram_tensor("ccx_in", [1, 1], f32)
    ccx_out = nc.dram_tensor("ccx_out", [ncores, 1], f32)

    wslt_v = wslt_ext[:].rearrange("(a p) n -> a p n", p=P)  # [KSL/P, P, N]
    wt_v = wt_ext[:].rearrange("(a p) n -> a p n", p=P)      # [KT, P, N]

    with tile.TileContext(nc) as tc:
        KC = min(2048, K)
        KHH = K // KC
        WCC = min(KC, N)
        with (
            tc.tile_pool(name="x4k", bufs=2) as xpool,       # [P, KC] f32 chunks
            tc.tile_pool(name="wq", bufs=3) as wpool,        # [P, 512] f32 chunks
            tc.tile_pool(name="qxc", bufs=2) as qxpool,      # [P, K] bf16
            tc.tile_pool(name="persist", bufs=1) as persist,
            tc.tile_pool(name="qwt", bufs=3) as qwtpool,
            tc.tile_pool(name="ob", bufs=3) as obpool,
            tc.tile_pool(name="stats", bufs=1) as stats,
            tc.tile_pool(name="psum_mm", bufs=6, space="PSUM") as psmm,
            tc.tile_pool(name="psum_x", bufs=2, space="PSUM") as psx,
        ):
            rg = [list(range(ncores))]

            def amax_exchange(part_vec, cc_in, cc_out, gat, gmax):
                nc.sync.dma_start(out=cc_in[:], in_=part_vec[0:1, :])
                nc.gpsimd.collective_compute(
                    "AllGather", mybir.AluOpType.bypass, replica_groups=rg,
                    ins=[cc_in[:].opt()], outs=[cc_out[:].opt()])
                nc.sync.dma_start(out=gat, in_=cc_out[:])
                nc.gpsimd.partition_all_reduce(gmax, gat, channels=ncores,
                                               reduce_op=bass_isa.ReduceOp.max)

            # ---------- Phase A2: x amax ----------
            xmaxes = stats.tile([P, MB * KHH], f32)
            for i in range(MB):
                for h in range(KHH):
                    xc = xpool.tile([P, KC], f32, tag="big")
                    nc.sync.dma_start(out=xc, in_=x_ext[i * P:(i + 1) * P, h * KC:(h + 1) * KC])
                    nc.vector.tensor_reduce(
                        out=xmaxes[:, i * KHH + h:i * KHH + h + 1], in_=xc,
                        axis=mybir.AxisListType.X, op=mybir.AluOpType.max,
                        apply_absolute_value=True)
            xmax_v = stats.tile([P, 1], f32)
            nc.vector.tensor_reduce(out=xmax_v, in_=xmaxes, axis=mybir.AxisListType.X,
                                    op=mybir.AluOpType.max)
            xmax_p = stats.tile([P, 1], f32)
            nc.gpsimd.partition_all_reduce(xmax_p, xmax_v, channels=P,
                                           reduce_op=bass_isa.ReduceOp.max)
            gat_x = stats.tile([ncores, 1], f32)
            gmax_x = stats.tile([ncores, 1], f32)
            amax_exchange(xmax_p, ccx_in, ccx_out, gat_x, gmax_x)
            ax = gmax_x[0:1, 0:1]

            # ---------- Phase A1: w-slice amax (small, first) ----------
            wmaxes = stats.tile([P, (KSL // P) * (N // WCC)], f32)
            for i in range(KSL // P):
                for h in range(N // WCC):
                    wcs = xpool.tile([P, KC], f32, tag="big")
                    nc.sync.dma_start(out=wcs[:, 0:WCC], in_=wslt_v[i, :, h * WCC:(h + 1) * WCC])
                    nc.vector.tensor_reduce(
                        out=wmaxes[:, i * (N // WCC) + h:i * (N // WCC) + h + 1],
                        in_=wcs[:, 0:WCC],
                        axis=mybir.AxisListType.X, op=mybir.AluOpType.max,
                        apply_absolute_value=True)
            wmax_v = stats.tile([P, 1], f32)
            nc.vector.tensor_reduce(out=wmax_v, in_=wmaxes, axis=mybir.AxisListType.X,
                                    op=mybir.AluOpType.max)
            wmax_p = stats.tile([P, 1], f32)
            nc.gpsimd.partition_all_reduce(wmax_p, wmax_v, channels=P,
                                           reduce_op=bass_isa.ReduceOp.max)
            gat_w = stats.tile([ncores, 1], f32)
            gmax_w = stats.tile([ncores, 1], f32)
            amax_exchange(wmax_p, ccw_in, ccw_out, gat_w, gmax_w)
            aw = gmax_w[0:1, 0:1]

            # ---------- scales ----------
            scw = stats.tile([1, 4], f32)
            scx = stats.tile([1, 4], f32)
            sx_t = stats.tile([1, 1], f32)
            sw_t = stats.tile([1, 1], f32)
            ds_t = stats.tile([1, 1], f32)
            dsc = stats.tile([1, 4], f32)

            def recip(dst, src, t0, t1):
                nc.vector.reciprocal(dst, src)
                nc.vector.tensor_tensor(out=t0, in0=src, in1=dst,
                                        op=mybir.AluOpType.mult)
                nc.vector.tensor_scalar(out=t1, in0=t0, scalar1=-1.0, scalar2=2.0,
                                        op0=mybir.AluOpType.mult,
                                        op1=mybir.AluOpType.add)
                nc.vector.tensor_tensor(out=dst, in0=dst, in1=t1,
                                        op=mybir.AluOpType.mult)

            recip(scx[0:1, 0:1], ax, scx[0:1, 1:2], scx[0:1, 2:3])
            nc.vector.tensor_scalar(out=sx_t, in0=scx[0:1, 0:1], scalar1=MAXV,
                                    scalar2=None, op0=mybir.AluOpType.mult)
            sxb = stats.tile([P, 1], f32)
            nc.gpsimd.partition_broadcast(sxb, sx_t)

            recip(scw[0:1, 0:1], aw, scw[0:1, 1:2], scw[0:1, 2:3])
            nc.vector.tensor_scalar(out=sw_t, in0=scw[0:1, 0:1], scalar1=MAXV,
                                    scalar2=None, op0=mybir.AluOpType.mult)
            swb = stats.tile([P, 1], f32)
            nc.gpsimd.partition_broadcast(swb, sw_t)

            nc.vector.tensor_tensor(out=dsc[0:1, 0:1], in0=sx_t, in1=sw_t,
                                    op=mybir.AluOpType.mult)
            recip(ds_t, dsc[0:1, 0:1], dsc[0:1, 1:2], dsc[0:1, 2:3])
            dsb = stats.tile([P, 1], f32)
            nc.gpsimd.partition_broadcast(dsb, ds_t)

            # bias replicated into all partitions (fp32)
            bias_t = persist.tile([P, N], bf16)
            bias_bcast = bass.AP(tensor=b_ext, offset=0, ap=[[0, P], [1, N]])
            nc.gpsimd.dma_start(out=bias_t, in_=bias_bcast)

            ident_b = persist.tile([P, P], bf16)
            make_identity(nc, ident_b[:])

            # ---------- Phase C: x quantize + on-chip transpose, per strip ----------
            qxT = persist.tile([P, KT, M], bf16)
            KTH = KC // P      # k-tiles per half-chunk
            for i in range(MB):
                for h in range(KHH):
                    xc = xpool.tile([P, KC], f32, tag="big")
                    nc.sync.dma_start(out=xc, in_=x_ext[i * P:(i + 1) * P, h * KC:(h + 1) * KC])
                    nc.vector.tensor_scalar(out=xc, in0=xc, scalar1=sxb,
                                            scalar2=MAGIC, op0=mybir.AluOpType.mult,
                                            op1=mybir.AluOpType.add)
                    qc = qxpool.tile([P, KC], bf16)
                    nc.scalar.activation(out=qc, in_=xc,
                                         func=mybir.ActivationFunctionType.Copy,
                                         bias=-MAGIC, scale=1.0)
                    for g in range(KTH // XG):
                        px = psx.tile([P, XG, P], bf16, space="PSUM")
                        for j in range(XG):
                            ktl = g * XG + j
                            nc.tensor.transpose(px[:, j, :], qc[:, ktl * P:(ktl + 1) * P],
                                                ident_b[:])
                        kt0 = h * KTH + g * XG
                        nc.scalar.copy(
                            out=qxT[:, kt0:kt0 + XG, i * P:(i + 1) * P],
                            in_=px[:])

            # ---------- Phase D: per-block w quantize (vector only) + matmul ----------
            for nb in range(NB):
                qwT = qwtpool.tile([P, KT, 512], bf16)
                for kt in range(KT):
                    wcs = wpool.tile([P, 512], f32)
                    nc.sync.dma_start(out=wcs,
                                      in_=wt_v[kt, :, nb * 512:(nb + 1) * 512])
                    nc.vector.tensor_scalar(out=wcs, in0=wcs, scalar1=swb,
                                            scalar2=MAGIC, op0=mybir.AluOpType.mult,
                                            op1=mybir.AluOpType.add)
                    nc.scalar.activation(out=qwT[:, kt, :], in_=wcs,
                                         func=mybir.ActivationFunctionType.Copy,
                                         bias=-MAGIC, scale=1.0)
                for mb in range(MB):
                    acc = psmm.tile([P, 512], f32, space="PSUM")
                    for kt in range(KT):
                        nc.tensor.matmul(
                            acc, qxT[:, kt, mb * P:(mb + 1) * P], qwT[:, kt, :],
                            start=(kt == 0), stop=(kt == KT - 1))
                    ob = obpool.tile([P, 512], f32)
                    nc.vector.scalar_tensor_tensor(
                        out=ob, in0=acc, scalar=dsb,
                        in1=bias_t[:, nb * 512:(nb + 1) * 512],
                        op0=mybir.AluOpType.mult, op1=mybir.AluOpType.add)
                    nc.sync.dma_start(
                        out=out_ext[mb * P:(mb + 1) * P, nb * 512:(nb + 1) * 512],
                        in_=ob)
    nc.compile()
    return nc


def shard_inputs(x, weight, bias, M=1024, K=4096, ncores=NCORES):
    xf = np.ascontiguousarray(np.asarray(x, dtype=np.float32).reshape(-1, x.shape[-1]))
    wT = np.ascontiguousarray(np.asarray(weight, dtype=np.float32).T)  # [K, N]
    b = np.ascontiguousarray(np.asarray(bias, dtype=np.float32))
    ksl = K // ncores
    in_maps = []
    for c in range(ncores):
        in_maps.append({
            "x": np.ascontiguousarray(xf[c * M:(c + 1) * M]),
            "wT": wT,
            "wslT": np.ascontiguousarray(wT[c * ksl:(c + 1) * ksl]),
            "bias": b,
        })
    return in_maps


def _run(x, weight, bias, trace=False):
    from concourse.bass_utils import run_bass_kernel_spmd

    nc = build_graph()
    in_maps = shard_inputs(x, weight, bias)
    res = run_bass_kernel_spmd(nc, in_maps, core_ids=list(range(NCORES)),
                               trace=trace)
    outs = [res.results[c]["out"] for c in range(NCORES)]
    full = np.concatenate(outs, axis=0).reshape(FULL_B, FULL_S, FULL_N)
    return full.astype(np.float32), res


def kernel(x, weight, bias):
    out, _ = _run(x, weight, bias, trace=False)
    return out
